# revision 15
# baseline (speedup 1.0000x reference)
"""GroupedQueryAttention on 8 TRN2 NeuronCores.

Sharding (hardcoded): core c -> batch b=c//4, head-group g=c%4.
Each core computes q-heads [g*8, g*8+8) and kv-heads [g*2, g*2+2)
(query heads co-located with their KV group), out_proj row-parallel
over heads (host sums the 4 partials per batch).

Shapes: B=2, S=1024, D_MODEL=2048, H=32, HKV=8, HEAD_DIM=64.

Device dataflow per core:
  A3: QT[512f,1024s]  = (Wq_c x_q^T) via f32r matmuls (feature-major)
  A1: KT[128f,1024s]  = (Wk_c x_k^T) f32r
  A2: V'[k,65]        = (x_v Wv_c^T | ones) bf16 (seq-major + ones col)
  B : per q-head: scoresT[k,q] = KT_h^T QT_h (f32r);
      un = exp(0.125*scoresT) (bf16); av = [V'|1]^T un (rows 0:64 = y^T,
      row 64 = softmax denominator); recip; K=1 broadcast matmul;
      attn = un * recip  -> DRAM [head, k, q] (host transposes to [q, k])
      y^T = av[0:64] * recip -> yt (bf16)
  C : out_partial = yt^T woT (bf16) -> DRAM [1024, 2048]
"""

import os
import sys
import numpy as np

for _p in ("/opt/trn_rl_repo", "/root/.axon_site/_ro/trn_rl_repo"):
    if os.path.isdir(_p) and _p not in sys.path:
        sys.path.insert(0, _p)

import ml_dtypes

B, S, D_MODEL = 2, 1024, 2048
NUM_HEADS, NUM_KV_HEADS, HEAD_DIM = 32, 8, 64
N_CORES = 8
HPC = NUM_HEADS // 4            # 8 q-heads per core
FQ = HPC * HEAD_DIM             # 512 q-features per core
FKV = 2 * HEAD_DIM              # 128 kv-features per core
NI = D_MODEL // 128             # 16 contraction chunks
BF16 = ml_dtypes.bfloat16

_CACHE = {}


def _build_program():
    import concourse.tile as tile
    from concourse import bacc, mybir

    F32 = mybir.dt.float32
    F32R = mybir.dt.float32r
    F16 = mybir.dt.float16
    EXP = mybir.ActivationFunctionType.Exp

    nc = bacc.Bacc("TRN2", target_bir_lowering=False, debug=False,
                   num_devices=N_CORES)

    xq_d = nc.dram_tensor("xq", [D_MODEL, S], F16, kind="ExternalInput").ap()
    xk_d = nc.dram_tensor("xk", [D_MODEL, S], F16, kind="ExternalInput").ap()
    xv_d = nc.dram_tensor("xv", [D_MODEL, S], F16, kind="ExternalInput").ap()
    wq_d = nc.dram_tensor("wq", [D_MODEL, FQ], F16, kind="ExternalInput").ap()
    wk_d = nc.dram_tensor("wk", [D_MODEL, FKV], F16, kind="ExternalInput").ap()
    wv_d = nc.dram_tensor("wv", [D_MODEL, FKV], F16, kind="ExternalInput").ap()
    wo_d = nc.dram_tensor("wo", [FQ, D_MODEL], F16, kind="ExternalInput").ap()
    bq_d = nc.dram_tensor("bq", [1, FQ], F16, kind="ExternalInput").ap()
    bk_d = nc.dram_tensor("bk", [1, FKV], F16, kind="ExternalInput").ap()
    bv_d = nc.dram_tensor("bv", [1, FKV], F16, kind="ExternalInput").ap()
    un_d = nc.dram_tensor("un_t", [HPC, S, S], F16,
                          kind="ExternalOutput").ap()
    rs_d = nc.dram_tensor("rs_t", [HPC, S], F32, kind="ExternalOutput").ap()
    outp_d = nc.dram_tensor("out_p", [S, D_MODEL], F16,
                            kind="ExternalOutput").ap()

    with tile.TileContext(nc, trace_sim=False) as tc:
        _emit(nc, tc, tile, mybir, locals())
    nc.compile()
    return nc


def _emit(nc, tc, tile, mybir, t):
    from contextlib import ExitStack

    F32 = mybir.dt.float32
    F32R = mybir.dt.float32r
    F16 = mybir.dt.float16
    EXP = mybir.ActivationFunctionType.Exp
    xq_d, xk_d, xv_d = t["xq_d"], t["xk_d"], t["xv_d"]
    wq_d, wk_d, wv_d, wo_d = t["wq_d"], t["wk_d"], t["wv_d"], t["wo_d"]
    bq_d, bk_d, bv_d = t["bq_d"], t["bk_d"], t["bv_d"]
    un_d, rs_d, outp_d = t["un_d"], t["rs_d"], t["outp_d"]

    ctx = ExitStack()
    with ctx:
        # ---------------- pools ----------------
        const = ctx.enter_context(tc.tile_pool(name="const", bufs=1))
        xqp = ctx.enter_context(tc.tile_pool(name="xqp", bufs=8))
        xkp = ctx.enter_context(tc.tile_pool(name="xkp", bufs=4))
        xvp = ctx.enter_context(tc.tile_pool(name="xvp", bufs=1))
        wpool = ctx.enter_context(tc.tile_pool(name="wpool", bufs=1))
        actp = ctx.enter_context(tc.tile_pool(name="actp", bufs=1))
        unp = ctx.enter_context(tc.tile_pool(name="unp", bufs=5))
        bcp = ctx.enter_context(tc.tile_pool(name="bcp", bufs=2))
        rsp = ctx.enter_context(tc.tile_pool(name="rsp", bufs=2))
        osp = ctx.enter_context(tc.tile_pool(name="osp", bufs=4))
        ps = ctx.enter_context(tc.tile_pool(name="ps", bufs=7, space="PSUM"))
        psv = ctx.enter_context(tc.tile_pool(name="psv", bufs=1, space="PSUM"))

        # ---------------- constants ----------------
        ones_row_f = const.tile([1, 512], F32, tag="ones_row_f")
        nc.vector.memset(ones_row_f[:], 1.0)
        ones_row = const.tile([1, 512], F16, tag="ones_row")
        nc.vector.tensor_copy(ones_row[:], ones_row_f[:])
        ones_row_bf = const.tile([1, 128], F16, tag="ones_row_bf")
        nc.vector.memset(ones_row_bf[:], 1.0)
        onecol_bf = const.tile([128, 1], F16, tag="onecol_bf")
        nc.vector.memset(onecol_bf[:], 1.0)
        ones1 = const.tile([1, 128], F32R, tag="ones1")
        nc.vector.tensor_copy(ones1[:], ones_row_f[:, 0:128])

        bq_sb = const.tile([1, FQ], F16, tag="bq")
        nc.sync.dma_start(bq_sb[:], bq_d[:])
        bk_sb = const.tile([1, FKV], F16, tag="bk")
        nc.sync.dma_start(bk_sb[:], bk_d[:])
        bv_sb = const.tile([1, FKV], F16, tag="bv")
        nc.sync.dma_start(bv_sb[:], bv_d[:])

        # persistent activations
        qt = actp.tile([128, 4, S], F16, tag="qt")      # [dpart, ftile, s]
        kt_sb = actp.tile([128, S], F16, tag="kt")      # [kvfeat, s]
        vp = [actp.tile([128, 8, 65], F16, tag=f"vp{h}", name=f"vp{h}")
              for h in range(2)]
        yt = actp.tile([128, 4, S], F16, tag="yt")      # [hdpart, htile, s]

        # ---------------- A3: Q projection ----------------
        wq_all = wpool.tile([128, NI, FQ], F16, tag="bigw")
        for i in range(NI):
            nc.sync.dma_start(wq_all[:, i, :], wq_d[i * 128:(i + 1) * 128, :])
        for sc in range(2):
            qps = [ps.tile([128, 512], F32, tag="p512", name=f"qps{sc}_{ft}")
                   for ft in range(4)]
            for i in range(NI):
                xqc = xqp.tile([128, 512], F16, tag="xq", name=f"xqc{sc}_{i}")
                nc.sync.dma_start(
                    xqc[:], xq_d[i * 128:(i + 1) * 128,
                                 sc * 512:(sc + 1) * 512])
                for ft in range(4):
                    nc.tensor.matmul(
                        qps[ft][:], wq_all[:, i, ft * 128:(ft + 1) * 128],
                        xqc[:], start=(i == 0), stop=False)
            for ft in range(4):
                nc.tensor.matmul(qps[ft][:], bq_sb[:, ft * 128:(ft + 1) * 128],
                                 ones_row[:], start=False, stop=True)
                nc.vector.tensor_copy(qt[:, ft, sc * 512:(sc + 1) * 512],
                                      qps[ft][:])

        # ---------------- A1: K projection ----------------
        wk_all = wpool.tile([128, NI, FKV], F16, tag="wk")
        for i in range(NI):
            nc.sync.dma_start(wk_all[:, i, :], wk_d[i * 128:(i + 1) * 128, :])
        kpsum = [ps.tile([128, 512], F32, tag="p512", name=f"kpsum{sc}")
                 for sc in range(2)]
        for i in range(NI):
            xkc = xkp.tile([128, S], F16, tag="xk")
            nc.sync.dma_start(xkc[:], xk_d[i * 128:(i + 1) * 128, :])
            for sc in range(2):
                nc.tensor.matmul(kpsum[sc][:], wk_all[:, i, :],
                                 xkc[:, sc * 512:(sc + 1) * 512],
                                 start=(i == 0), stop=False)
        for sc in range(2):
            nc.tensor.matmul(kpsum[sc][:], bk_sb[:], ones_row[:],
                             start=False, stop=True)
            nc.vector.tensor_copy(kt_sb[:, sc * 512:(sc + 1) * 512], kpsum[sc][:])

        # ---------------- A2: V projection (seq-major, bf16) ----------------
        wv_all = wpool.tile([128, NI, FKV], F16, tag="wv")
        for i in range(NI):
            nc.sync.dma_start(wv_all[:, i, :], wv_d[i * 128:(i + 1) * 128, :])
        xv_all = xvp.tile([128, NI, S], F16, tag="xv")
        for i in range(NI):
            nc.sync.dma_start(xv_all[:, i, :], xv_d[i * 128:(i + 1) * 128, :])
        for kt8 in range(8):
            pv = psv.tile([128, 128], F32, tag="p128")
            for i in range(NI):
                nc.tensor.matmul(pv[:],
                                 xv_all[:, i, kt8 * 128:(kt8 + 1) * 128],
                                 wv_all[:, i, :], start=(i == 0), stop=False)
            nc.tensor.matmul(pv[:], ones_row_bf[:], bv_sb[:],
                             start=False, stop=True)
            for h in range(2):
                nc.vector.tensor_copy(vp[h][:, kt8, 0:64],
                                      pv[:, h * 64:(h + 1) * 64])
                nc.vector.tensor_copy(vp[h][:, kt8, 64:65], onecol_bf[:])

        # ---------------- B: attention per local q-head ----------------
        for l in range(HPC):
            kv = l // 4
            # host orders q-features so head l sits in tile l%4 at partition
            # offset (l//4)*64 == its kv head's offset in kt_sb (matmul
            # requires equal base partitions for lhsT and rhs)
            pq, pof = l % 4, kv * 64
            av_sb = rsp.tile([65, S], F32, tag="av_sb")
            for qc in range(2):
                un = unp.tile([128, 8, 512], F16, tag="un", name=f"un{l}_{qc}")
                av = ps.tile([65, 512], F32, tag="p512")
                for kt8 in range(8):
                    sc_ps = ps.tile([128, 512], F32, tag="p512")
                    nc.tensor.matmul(
                        sc_ps[:],
                        kt_sb[kv * 64:(kv + 1) * 64, kt8 * 128:(kt8 + 1) * 128],
                        qt[pof:pof + 64, pq, qc * 512:(qc + 1) * 512],
                        start=True, stop=True)
                    nc.scalar.activation(un[:, kt8, :], sc_ps[:], EXP,
                                         scale=0.125)
                    nc.tensor.matmul(av[:], vp[kv][:, kt8, :], un[:, kt8, :],
                                     start=(kt8 == 0), stop=(kt8 == 7))
                # unnormalized exp straight to DRAM (host normalizes)
                nc.sync.dma_start(
                    un_d[l].rearrange("(t p) q -> p t q", p=128)[
                        :, :, qc * 512:(qc + 1) * 512], un[:, :, :])
                nc.vector.tensor_copy(av_sb[:, qc * 512:(qc + 1) * 512], av[:])
            nc.sync.dma_start(rs_d[l:l + 1, :], av_sb[64:65, :])
            rcp = rsp.tile([1, S], F32R, tag="rcp")
            with nc.allow_low_precision(reason="f32r recip feeds bcast matmul"):
                nc.vector.reciprocal(rcp[:], av_sb[64:65, :])
            bc_sb = bcp.tile([64, S], F32, tag="bc")
            for qc in range(2):
                bc_ps = ps.tile([64, 512], F32, tag="p512")
                nc.tensor.matmul(bc_ps[:], ones1[:, 0:64],
                                 rcp[:, qc * 512:(qc + 1) * 512],
                                 start=True, stop=True)
                nc.vector.tensor_copy(bc_sb[:, qc * 512:(qc + 1) * 512],
                                      bc_ps[:])
            for qc in range(2):
                nc.vector.tensor_mul(yt[pof:pof + 64, pq, qc * 512:(qc + 1) * 512],
                                     av_sb[0:64, qc * 512:(qc + 1) * 512],
                                     bc_sb[:, qc * 512:(qc + 1) * 512])

        # ---------------- C: out projection (bf16, row-parallel) ----------
        wo_all = wpool.tile([128, 4, D_MODEL], F16, tag="bigw")
        for tt in range(4):
            nc.sync.dma_start(wo_all[:, tt, :], wo_d[tt * 128:(tt + 1) * 128, :])
        for st in range(8):
            for fc in range(4):
                p = ps.tile([128, 512], F32, tag="p512")
                for tt in range(4):
                    nc.tensor.matmul(p[:],
                                     yt[:, tt, st * 128:(st + 1) * 128],
                                     wo_all[:, tt, fc * 512:(fc + 1) * 512],
                                     start=(tt == 0), stop=(tt == 3))
                os_t = osp.tile([128, 512], F16, tag="os")
                nc.vector.tensor_copy(os_t[:], p[:])
                nc.sync.dma_start(
                    outp_d[st * 128:(st + 1) * 128,
                           fc * 512:(fc + 1) * 512], os_t[:])


def kernel(query, key, value, Wq, bq, Wk, bk, Wv, bv, Wo, bo):
    from concourse.bass_utils import run_bass_kernel_spmd

    query = np.asarray(query, dtype=np.float32)
    key = np.asarray(key, dtype=np.float32)
    value = np.asarray(value, dtype=np.float32)
    Wq = np.asarray(Wq, dtype=np.float32)
    Wk = np.asarray(Wk, dtype=np.float32)
    Wv = np.asarray(Wv, dtype=np.float32)
    Wo = np.asarray(Wo, dtype=np.float32)
    bq = np.asarray(bq, dtype=np.float32)
    bk = np.asarray(bk, dtype=np.float32)
    bv = np.asarray(bv, dtype=np.float32)
    bo = np.asarray(bo, dtype=np.float32)

    if "nc" not in _CACHE:
        _CACHE["nc"] = _build_program()
    nc = _CACHE["nc"]

    # q-feature permutation: head l -> qt tile l%4, partition offset (l//4)*64
    head_order = [0, 4, 1, 5, 2, 6, 3, 7]
    feat_idx = np.concatenate(
        [np.arange(h * HEAD_DIM, (h + 1) * HEAD_DIM) for h in head_order])

    in_maps = []
    for c in range(N_CORES):
        b, g = c // 4, c % 4
        fq, fkv0 = g * FQ + feat_idx, g * FKV
        in_maps.append({
            "xq": np.ascontiguousarray(query[b].T.astype(np.float16)),
            "xk": np.ascontiguousarray(key[b].T.astype(np.float16)),
            "xv": np.ascontiguousarray(value[b].T.astype(np.float16)),
            "wq": np.ascontiguousarray(Wq[fq, :].T.astype(np.float16)),
            "wk": np.ascontiguousarray(Wk[fkv0:fkv0 + FKV, :].T.astype(np.float16)),
            "wv": np.ascontiguousarray(Wv[fkv0:fkv0 + FKV, :].T.astype(np.float16)),
            "wo": np.ascontiguousarray(Wo[:, fq].T.astype(np.float16)),
            "bq": bq[fq][None, :].astype(np.float16),
            "bk": bk[fkv0:fkv0 + FKV][None, :].astype(np.float16),
            "bv": bv[fkv0:fkv0 + FKV][None, :].astype(np.float16),
        })

    res = run_bass_kernel_spmd(nc, in_maps, core_ids=list(range(N_CORES)))

    out = np.zeros((B, S, D_MODEL), dtype=np.float32)
    attn = np.empty((B, NUM_HEADS, S, S), dtype=np.float32)
    for c in range(N_CORES):
        b, g = c // 4, c % 4
        out[b] += res.results[c]["out_p"].astype(np.float32)
        un = res.results[c]["un_t"].astype(np.float32)   # [l, k, q] unnorm
        un /= res.results[c]["rs_t"][:, None, :]         # softmax denom per q
        attn[b, g * HPC:(g + 1) * HPC] = un.transpose(0, 2, 1)
    out += bo[None, None, :]
    return out, attn
